# revision 6
# baseline (speedup 1.0000x reference)
"""RWKV-4 block (nn_Block_5669356833485) Trainium2 Bass kernel, v2.

B=8, T=2048, C=1024, HID=4096. B-sharded across 8 NeuronCores (1 batch/core).
Feature-major layout [C-partitions, T-free].

v2 strategy (vs v1 baseline at 1.19ms):
- fp8e4m3 DoubleRow matmuls (0.5 cyc/row, K=256/instr) for k/v/r (lerp folded
  into paired weights vs z and shifted z), Wo (sry pairs), and fWv (kk pairs).
- fWk/fWr stay bf16 (relu^2 amplifies fp8 error too much on that path).
- WKV scan in f32 (scans get no bf16 speedup anyway), one reciprocal per cb.
- kk = relu(k)^2 fused into ONE scalar_tensor_tensor (max then mult) with the
  fp8 scale folded into fWk (sqrt) and fWv weights.
- bf16 residual streams (xT, x2) and epilogues; per-m software pipeline so
  WKV (DVE/Pool) overlaps projections (PE).
Self-contained: hardcodes shapes; no sibling imports.
"""
import contextlib
import os
import sys
sys.path.insert(0, '/opt/trn_rl_repo')

KPHASES = int(os.environ.get("KPHASES", "99"))
KP5 = int(os.environ.get("KP5", "99"))

import numpy as np
import ml_dtypes

import concourse.bass as bass
from concourse import bacc
import concourse.mybir as mybir
import concourse.tile as tile
from concourse.bass_utils import run_bass_kernel_spmd

F32 = mybir.dt.float32
F32R = mybir.dt.float32r
BF16 = mybir.dt.bfloat16
F8 = mybir.dt.float8e4
AL = mybir.AluOpType
AF = mybir.ActivationFunctionType
DR = mybir.MatmulPerfMode.DoubleRow

B, T, C, HID = 8, 2048, 1024, 4096
NCB = C // 128           # 8 channel blocks
NHB = HID // 128         # 32 hidden blocks
NT = T // 512            # 4 n-slices of 512
TH = T // 2              # half length 1024
ZPAD = T + 4             # zPair free stride (col0 = shift pad, 4-aligned)
EPS = 1e-5

# fp8 scales (host side); dequants live in cst columns
SZ = 1.0       # z fp8 scale (z ~ N(0,1) fits e4m3 range directly)
SS = 32.0      # sry fp8 scale
SKK = 16.0     # kk fp8 scale (sqrt folded into fWk)
WMAX = 192.0   # weight fp8 absmax target

# cst columns
C_WBAR, C_EU, C_BK, C_BR, C_BFR, C_FTM, C_FTM1 = 0, 8, 16, 24, 32, 40, 48
C_EPS, C_ZERO, C_CV, C_CO, C_CFV, C_CK, C_CR = 56, 57, 58, 59, 60, 61, 62
NCOLS = 63


def _emit(nc):
    x_d = nc.declare_dram_parameter("x", [T, C], F32, isOutput=False)
    wk_d = nc.declare_dram_parameter("wk8", [128, 16384], F8, isOutput=False)
    wv_d = nc.declare_dram_parameter("wv8", [128, 16384], F8, isOutput=False)
    wr_d = nc.declare_dram_parameter("wr8", [128, 16384], F8, isOutput=False)
    wo_d = nc.declare_dram_parameter("wo8", [128, 8192], F8, isOutput=False)
    fwk_d = nc.declare_dram_parameter("fwk16", [128, NHB * NCB * 128], BF16,
                                      isOutput=False)
    fwr_d = nc.declare_dram_parameter("fwr16", [128, NCB * NCB * 128], BF16,
                                      isOutput=False)
    fwv_d = nc.declare_dram_parameter("fwv8", [128, 32768], F8, isOutput=False)
    cst_d = nc.declare_dram_parameter("cst", [128, NCOLS], F32, isOutput=False)
    identb_d = nc.declare_dram_parameter("identb", [128, 128], BF16,
                                         isOutput=False)
    ones1_d = nc.declare_dram_parameter("ones1b", [128, 1], BF16, isOutput=False)
    onesb_d = nc.declare_dram_parameter("onesbr", [1, 128], F32R, isOutput=False)
    out_d = nc.declare_dram_parameter("out", [T, C], F32, isOutput=True)

    with tile.TileContext(nc, pool_alloc_mode="queue") as tc:
      with tc.tile_pool(name="pc", bufs=1) as pc:
        cst = pc.tile([128, NCOLS], F32)
        nc.sync.dma_start(out=cst, in_=cst_d[:])
        identb = pc.tile([128, 128], BF16)
        nc.sync.dma_start(out=identb, in_=identb_d[:])
        ones1 = pc.tile([128, 1], BF16)
        nc.sync.dma_start(out=ones1, in_=ones1_d[:])
        onesb = pc.tile([1, 128], F32R)
        nc.sync.dma_start(out=onesb, in_=onesb_d[:])

        def col(j):
            return cst[:, j:j + 1]

        # LEFT long-lived pools, created in reverse-close order (LIFO):
        # close order: st1(P1e) zp(P2e) wtm(P2e) sry(P3e) xT(P3e) wo(P3e)
        es_wo = contextlib.ExitStack()
        p_wo = es_wo.enter_context(tc.tile_pool(name="p_wo", bufs=1))
        wo8 = p_wo.tile([128, 8192], F8)
        nc.sync.dma_start(out=wo8, in_=wo_d[:])
        es_xT = contextlib.ExitStack()
        p_xT = es_xT.enter_context(tc.tile_pool(name="p_xT", bufs=1))
        xT = [p_xT.tile([128, T], BF16, tag=f"xT{cb}", name=f"xT{cb}")
              for cb in range(NCB)]
        es_sry = contextlib.ExitStack()
        p_sry = es_sry.enter_context(tc.tile_pool(name="p_sry", bufs=1))
        sryp = [p_sry.tile([128, 2, T], F8, tag=f"sry{j}", name=f"sry{j}")
                for j in range(4)]
        es_wtm = contextlib.ExitStack()
        p_wtm = es_wtm.enter_context(tc.tile_pool(name="p_wtm", bufs=1))
        wk8 = p_wtm.tile([128, 16384], F8, tag="wk")
        wv8 = p_wtm.tile([128, 16384], F8, tag="wv")
        wr8 = p_wtm.tile([128, 16384], F8, tag="wr")
        nc.sync.dma_start(out=wk8, in_=wk_d[:])
        nc.sync.dma_start(out=wv8, in_=wv_d[:])
        nc.sync.dma_start(out=wr8, in_=wr_d[:])
        es_zp = contextlib.ExitStack()
        p_zp = es_zp.enter_context(tc.tile_pool(name="p_zp", bufs=1))
        zp = [p_zp.tile([128, 2, ZPAD], F8, tag=f"zp{j}", name=f"zp{j}")
              for j in range(4)]
        es_st1 = contextlib.ExitStack()
        p_st1 = es_st1.enter_context(tc.tile_pool(name="p_st1", bufs=1))
        mean_sb = p_st1.tile([1, T], F32, tag="mean")
        msq_sb = p_st1.tile([1, T], F32, tag="msq")

        def wpair(wsb, b):
            """[128, 2, 128] DR pair slice at block index b."""
            return wsb[:, b * 256:(b + 1) * 256].rearrange(
                "p (i q) -> p i q", i=2)

        # ---------------- P0: load x (cast bf16), transpose, LN1 stats -----
        with tc.tile_pool(name="p_ld", bufs=5) as p_ld, \
             tc.tile_pool(name="ps_tr", bufs=3, space="PSUM") as ps_tr, \
             tc.tile_pool(name="ps_st", bufs=2, space="PSUM") as ps_st, \
             tc.tile_pool(name="p_sq", bufs=3) as p_sq:
            for tbg in range(4):
                xt = []
                for j in range(4):
                    tb = tbg * 4 + j
                    t_ = p_ld.tile([128, C], BF16, tag="xtok")
                    nc.gpsimd.dma_start(
                        out=t_, in_=x_d[tb * 128:(tb + 1) * 128, :])
                    xt.append(t_)
                for cb in range(NCB):
                    pt = ps_tr.tile([128, 512], BF16, tag="tr")
                    for j in range(4):
                        nc.tensor.transpose(
                            pt[:, j * 128:(j + 1) * 128],
                            xt[j][:, cb * 128:(cb + 1) * 128], identb)
                    dst = xT[cb][:, tbg * 512:(tbg + 1) * 512]
                    if cb % 2 == 0:
                        nc.vector.tensor_copy(out=dst, in_=pt)
                    else:
                        nc.scalar.copy(dst, pt)
                # LN1 stats for this 512-token slice
                sl = slice(tbg * 512, (tbg + 1) * 512)
                mps = ps_st.tile([1, 512], F32, tag="mps")
                qps = ps_st.tile([1, 512], F32, tag="qps")
                for cb in range(NCB):
                    sq = p_sq.tile([128, 512], BF16, tag="sq")
                    nc.scalar.activation(sq, xT[cb][:, sl], AF.Square)
                    nc.tensor.matmul(mps, ones1, xT[cb][:, sl],
                                     start=(cb == 0), stop=(cb == NCB - 1))
                    nc.tensor.matmul(qps, ones1, sq,
                                     start=(cb == 0), stop=(cb == NCB - 1))
                nc.scalar.mul(mean_sb[:, sl], mps, 1.0 / C)
                nc.scalar.mul(msq_sb[:, sl], qps, 1.0 / C)

        # -------------- P1: LN1 globals + z (fp8 pairs) --------------------
        with tc.tile_pool(name="p_ln", bufs=1) as p_ln, \
             tc.tile_pool(name="p_u", bufs=3) as p_u, \
             tc.tile_pool(name="ps_bc", bufs=2, space="PSUM") as ps_bc:
            var = p_ln.tile([1, T], F32, tag="var")
            nc.vector.tensor_mul(var, mean_sb, mean_sb)
            nc.vector.tensor_sub(var, msq_sb, var)
            ve_b = p_ln.tile([128, T], F32, tag="ve")
            mean_b = p_ln.tile([128, T], BF16, tag="meanb")
            for n in range(NT):
                sl = slice(n * 512, (n + 1) * 512)
                bc = ps_bc.tile([128, 512], F32, tag="bc")
                nc.tensor.matmul(bc, onesb, var.bitcast(F32R)[:, sl],
                                 start=True, stop=True)
                nc.scalar.activation(ve_b[:, sl], bc, AF.Identity,
                                     bias=col(C_EPS))
                bc2 = ps_bc.tile([128, 512], F32, tag="bc")
                nc.tensor.matmul(bc2, onesb, mean_sb.bitcast(F32R)[:, sl],
                                 start=True, stop=True)
                if n % 2 == 0:
                    nc.vector.tensor_copy(out=mean_b[:, sl], in_=bc2)
                else:
                    nc.gpsimd.tensor_copy(out=mean_b[:, sl], in_=bc2)
            nc.vector.reciprocal(out=ve_b, in_=ve_b)
            rstd_b = p_ln.tile([128, T], BF16, tag="rstd")
            nc.scalar.activation(rstd_b, ve_b, AF.Sqrt)
            for j in range(4):
                nc.vector.memset(zp[j][:, :, 0:1], 0.0)
            for cb in range(NCB):
                u = p_u.tile([128, T], BF16, tag="u")
                if cb % 2 == 0:
                    nc.vector.tensor_sub(u, xT[cb], mean_b)
                    nc.gpsimd.tensor_mul(zp[cb // 2][:, cb % 2, 1:T + 1], u,
                                         rstd_b)
                else:
                    nc.gpsimd.tensor_sub(u, xT[cb], mean_b)
                    nc.vector.tensor_mul(zp[cb // 2][:, cb % 2, 1:T + 1], u,
                                         rstd_b)
        es_st1.close()

        # ---------------- P2: per-m projections + WKV ----------------------
        with tc.tile_pool(name="p_eksr", bufs=2) as p_eksr, \
             tc.tile_pool(name="p_wkv", bufs=2) as p_wkv, \
             tc.tile_pool(name="ps_p", bufs=4, space="PSUM") as ps_p:
            for m in (range(NCB) if KPHASES >= 2 else ()):
                ek = p_eksr.tile([128, T], BF16, tag="ek", name=f"ek{m}")
                sr = p_eksr.tile([128, T], BF16, tag="sr", name=f"sr{m}")
                ekv = p_wkv.tile([128, T], BF16, tag="ekv", name=f"ekv{m}")
                A = p_wkv.tile([128, T + 1], F32, tag="A", name=f"A{m}")
                Bt = p_wkv.tile([128, T + 1], F32, tag="Bt", name=f"Bt{m}")
                srnum = p_wkv.tile([128, T], BF16, tag="srn", name=f"srn{m}")

                def proj(wsb, m):
                    ps = []
                    for n in range(NT):
                        p_ = ps_p.tile([128, 512], F32, tag="pp")
                        for j in range(4):
                            for s in range(2):
                                nc.tensor.matmul(
                                    p_, wpair(wsb, (j * 8 + m) * 2 + s),
                                    zp[j][:, :, (1 - s) + n * 512:
                                          (1 - s) + n * 512 + 512],
                                    start=(j == 0 and s == 0),
                                    stop=(j == 3 and s == 1), perf_mode=DR)
                        ps.append(p_)
                    return ps

                for n, p_ in enumerate(proj(wk8, m)):
                    nc.scalar.activation(
                        ek[:, n * 512:(n + 1) * 512], p_, AF.Exp,
                        bias=col(C_BK + m), scale=col(C_CK))
                for n, p_ in enumerate(proj(wv8, m)):
                    sl = slice(n * 512, (n + 1) * 512)
                    e1 = nc.vector if (m + n) % 2 == 0 else nc.gpsimd
                    e1.scalar_tensor_tensor(
                        out=ekv[:, sl], in0=p_, scalar=col(C_CV),
                        in1=ek[:, sl], op0=AL.mult, op1=AL.mult)
                for n, p_ in enumerate(proj(wr8, m)):
                    nc.scalar.activation(
                        sr[:, n * 512:(n + 1) * 512], p_, AF.Sigmoid,
                        bias=col(C_BR + m), scale=col(C_CR))
                # WKV
                E1 = nc.vector if m % 2 == 0 else nc.gpsimd
                E2 = nc.gpsimd if m % 2 == 0 else nc.vector
                nc.vector.memset(A[:, 0:1], 0.0)
                nc.vector.memset(Bt[:, 0:1], 0.0)
                wb = col(C_WBAR + m).broadcast_to([128, T])
                E1.tensor_tensor_scan(out=A[:, 1:T + 1], data0=wb, data1=ekv,
                                      initial=0.0, op0=AL.mult, op1=AL.add)
                E2.tensor_tensor_scan(out=Bt[:, 1:T + 1], data0=wb, data1=ek,
                                      initial=0.0, op0=AL.mult, op1=AL.add)
                E1.scalar_tensor_tensor(out=A[:, 0:T], in0=ekv,
                                        scalar=col(C_EU + m), in1=A[:, 0:T],
                                        op0=AL.mult, op1=AL.add)
                E2.scalar_tensor_tensor(out=Bt[:, 0:T], in0=ek,
                                        scalar=col(C_EU + m), in1=Bt[:, 0:T],
                                        op0=AL.mult, op1=AL.add)
                nc.vector.reciprocal(out=Bt[:, 0:T], in_=Bt[:, 0:T])
                E2.tensor_mul(srnum, sr, A[:, 0:T])
                E1.tensor_mul(sryp[m // 2][:, m % 2, :], srnum, Bt[:, 0:T])
        es_zp.close()
        es_wtm.close()

        # RIGHT long-lived pools: close order st2(P4e) z2(P5e) x2(P5e) wff(end)
        es_wff = contextlib.ExitStack()
        p_wff = es_wff.enter_context(
            tc.tile_pool(name="p_wff", bufs=1, side="right"))
        fwv8 = p_wff.tile([128, 32768], F8, tag="fwv")
        if KPHASES >= 3:
            nc.sync.dma_start(out=fwv8, in_=fwv_d[:])
        es_x2 = contextlib.ExitStack()
        p_x2 = es_x2.enter_context(
            tc.tile_pool(name="p_x2", bufs=1, side="right"))
        x2 = [p_x2.tile([128, T], BF16, tag=f"x2_{m}", name=f"x2_{m}")
              for m in range(NCB)]
        es_z2 = contextlib.ExitStack()
        p_z2 = es_z2.enter_context(
            tc.tile_pool(name="p_z2", bufs=1, side="right"))
        z2 = [p_z2.tile([128, T + 1], BF16, tag=f"z2_{cb}", name=f"z2_{cb}")
              for cb in range(NCB)]
        es_st2 = contextlib.ExitStack()
        p_st2 = es_st2.enter_context(
            tc.tile_pool(name="p_st2", bufs=1, side="right"))
        mean2_sb = p_st2.tile([1, T], F32, tag="mean2")
        msq2_sb = p_st2.tile([1, T], F32, tag="msq2")

        # ------------- P3: out-proj + x2 + LN2 stats -----------------------
        with tc.tile_pool(name="ps_o", bufs=4, space="PSUM") as ps_o, \
             tc.tile_pool(name="ps_st2", bufs=2, space="PSUM") as ps_st2, \
             tc.tile_pool(name="p_sq2", bufs=3) as p_sq2:
            for n in (range(NT) if KPHASES >= 3 else ()):
                sl = slice(n * 512, (n + 1) * 512)
                mps = ps_st2.tile([1, 512], F32, tag="mps2")
                qps = ps_st2.tile([1, 512], F32, tag="qps2")
                for m2 in range(NCB):
                    ps = ps_o.tile([128, 512], F32, tag="po")
                    for j2 in range(4):
                        nc.tensor.matmul(ps, wpair(wo8, j2 * 8 + m2),
                                         sryp[j2][:, :, sl],
                                         start=(j2 == 0), stop=(j2 == 3),
                                         perf_mode=DR)
                    e1 = nc.vector if (n + m2) % 2 == 0 else nc.gpsimd
                    e1.scalar_tensor_tensor(
                        out=x2[m2][:, sl], in0=ps, scalar=col(C_CO),
                        in1=xT[m2][:, sl], op0=AL.mult, op1=AL.add)
                    sq = p_sq2.tile([128, 512], BF16, tag="sq2")
                    nc.scalar.activation(sq, x2[m2][:, sl], AF.Square)
                    nc.tensor.matmul(mps, ones1, x2[m2][:, sl],
                                     start=(m2 == 0), stop=(m2 == NCB - 1))
                    nc.tensor.matmul(qps, ones1, sq,
                                     start=(m2 == 0), stop=(m2 == NCB - 1))
                nc.scalar.mul(mean2_sb[:, sl], mps, 1.0 / C)
                nc.scalar.mul(msq2_sb[:, sl], qps, 1.0 / C)
        es_sry.close()
        es_xT.close()
        es_wo.close()

        # ------------- P4: LN2 globals + z2 (bf16) -------------------------
        with tc.tile_pool(name="p_ln2", bufs=1) as p_ln2, \
             tc.tile_pool(name="p_u2", bufs=3) as p_u2, \
             tc.tile_pool(name="ps_bc2", bufs=2, space="PSUM") as ps_bc2:
            if KPHASES >= 4:
                var2 = p_ln2.tile([1, T], F32, tag="var2")
                nc.vector.tensor_mul(var2, mean2_sb, mean2_sb)
                nc.vector.tensor_sub(var2, msq2_sb, var2)
                ve2_b = p_ln2.tile([128, T], F32, tag="ve2")
                mean2_b = p_ln2.tile([128, T], BF16, tag="mean2b")
                for n in range(NT):
                    sl = slice(n * 512, (n + 1) * 512)
                    bc = ps_bc2.tile([128, 512], F32, tag="bc2")
                    nc.tensor.matmul(bc, onesb, var2.bitcast(F32R)[:, sl],
                                     start=True, stop=True)
                    nc.scalar.activation(ve2_b[:, sl], bc, AF.Identity,
                                         bias=col(C_EPS))
                    bc2_ = ps_bc2.tile([128, 512], F32, tag="bc2")
                    nc.tensor.matmul(bc2_, onesb,
                                     mean2_sb.bitcast(F32R)[:, sl],
                                     start=True, stop=True)
                    if n % 2 == 0:
                        nc.vector.tensor_copy(out=mean2_b[:, sl], in_=bc2_)
                    else:
                        nc.gpsimd.tensor_copy(out=mean2_b[:, sl], in_=bc2_)
                nc.vector.reciprocal(out=ve2_b, in_=ve2_b)
                rstd2_b = p_ln2.tile([128, T], BF16, tag="rstd2")
                nc.scalar.activation(rstd2_b, ve2_b, AF.Sqrt)
                for cb in range(NCB):
                    nc.vector.memset(z2[cb][:, 0:1], 0.0)
                    u = p_u2.tile([128, T], BF16, tag="u2")
                    if cb % 2 == 0:
                        nc.vector.tensor_sub(u, x2[cb], mean2_b)
                        nc.gpsimd.tensor_mul(z2[cb][:, 1:T + 1], u, rstd2_b)
                    else:
                        nc.gpsimd.tensor_sub(u, x2[cb], mean2_b)
                        nc.vector.tensor_mul(z2[cb][:, 1:T + 1], u, rstd2_b)
        es_st2.close()

        # --------------------- P5: FFN in two T-halves ---------------------
        with tc.tile_pool(name="p_fwk", bufs=3) as p_fwk, \
             tc.tile_pool(name="p_fwr", bufs=3) as p_fwr, \
             tc.tile_pool(name="p_stage", bufs=3) as p_stage:
            for h in (range(2) if KPHASES >= 5 else ()):
                hs0 = h * TH          # shifted slice start (col space)
                with tc.tile_pool(name=f"p_xf{h}", bufs=1) as p_xf, \
                     tc.tile_pool(name=f"p_t1{h}", bufs=3) as p_t1, \
                     tc.tile_pool(name=f"p_srf{h}", bufs=1) as p_srf, \
                     tc.tile_pool(name=f"p_kk{h}", bufs=1) as p_kk:
                    xf = [p_xf.tile([128, TH], BF16, tag=f"xf{cb}",
                                    name=f"xf{cb}_{h}") for cb in range(NCB)]
                    for cb in range(NCB):
                        t1 = p_t1.tile([128, TH], BF16, tag="t1")
                        nc.scalar.mul(t1, z2[cb][:, hs0:hs0 + TH],
                                      col(C_FTM1 + cb))
                        e1 = nc.vector if cb % 2 == 0 else nc.gpsimd
                        e1.scalar_tensor_tensor(
                            out=xf[cb], in0=z2[cb][:, hs0 + 1:hs0 + TH + 1],
                            scalar=col(C_FTM + cb), in1=t1,
                            op0=AL.mult, op1=AL.add)
                    srf = [p_srf.tile([128, TH], BF16, tag=f"srf{m2}",
                                      name=f"srf{m2}_{h}") for m2 in range(NCB)]
                    # fWr (bf16) -> sigmoid -> srf
                    with tc.tile_pool(name=f"ps_f{h}", bufs=2,
                                      space="PSUM") as ps_f:
                        for m2 in (range(NCB) if KP5 >= 2 else ()):
                            wfr = p_fwr.tile([128, 1024], BF16, tag="wfr")
                            nc.sync.dma_start(
                                out=wfr,
                                in_=fwr_d[:, m2 * 1024:(m2 + 1) * 1024])
                            for n2 in range(2):
                                ps = ps_f.tile([128, 512], F32, tag="pfr")
                                for cb in range(NCB):
                                    nc.tensor.matmul(
                                        ps, wfr[:, cb * 128:(cb + 1) * 128],
                                        xf[cb][:, n2 * 512:(n2 + 1) * 512],
                                        start=(cb == 0), stop=(cb == NCB - 1))
                                nc.scalar.activation(
                                    srf[m2][:, n2 * 512:(n2 + 1) * 512], ps,
                                    AF.Sigmoid, bias=col(C_BFR + m2))
                    # fWk (bf16) -> kk fp8 pairs
                    kkp = [p_kk.tile([128, 2, TH], F8, tag=f"kk{hp}",
                                     name=f"kk{hp}_{h}")
                           for hp in range(NHB // 2)]
                    with tc.tile_pool(name=f"ps_k{h}", bufs=3,
                                      space="PSUM") as ps_k:
                        for hb in (range(NHB) if KP5 >= 3 else ()):
                            wfk = p_fwk.tile([128, 1024], BF16, tag="wfk")
                            nc.sync.dma_start(
                                out=wfk,
                                in_=fwk_d[:, hb * 1024:(hb + 1) * 1024])
                            for n2 in range(2):
                                ps = ps_k.tile([128, 512], F32, tag="pkk")
                                for cb in range(NCB):
                                    nc.tensor.matmul(
                                        ps, wfk[:, cb * 128:(cb + 1) * 128],
                                        xf[cb][:, n2 * 512:(n2 + 1) * 512],
                                        start=(cb == 0), stop=(cb == NCB - 1))
                                e1 = (nc.vector if (hb + n2) % 2 == 0
                                      else nc.gpsimd)
                                e1.scalar_tensor_tensor(
                                    out=kkp[hb // 2][:, hb % 2,
                                                     n2 * 512:(n2 + 1) * 512],
                                    in0=ps, scalar=col(C_ZERO), in1=ps,
                                    op0=AL.max, op1=AL.mult)
                    # fWv (fp8 DR) -> rkv -> fin -> transpose -> out
                    with tc.tile_pool(name=f"p_fin{h}", bufs=2) as p_fin, \
                         tc.tile_pool(name=f"ps_v{h}", bufs=2,
                                      space="PSUM") as ps_v, \
                         tc.tile_pool(name=f"ps_t{h}", bufs=2,
                                      space="PSUM") as ps_t:
                        for n2 in (range(2) if KP5 >= 4 else ()):
                            fins = []
                            for m2 in range(NCB):
                                ps = ps_v.tile([128, 512], F32, tag="pv")
                                for hp in range(NHB // 2):
                                    nc.tensor.matmul(
                                        ps, wpair(fwv8, hp * 8 + m2),
                                        kkp[hp][:, :, n2 * 512:(n2 + 1) * 512],
                                        start=(hp == 0), stop=(hp == 15),
                                        perf_mode=DR)
                                rkv = p_fin.tile([128, 512], BF16, tag="rkv")
                                e1 = nc.vector if m2 % 2 == 0 else nc.gpsimd
                                e2 = nc.gpsimd if m2 % 2 == 0 else nc.vector
                                e1.scalar_tensor_tensor(
                                    out=rkv, in0=ps, scalar=col(C_CFV),
                                    in1=srf[m2][:, n2 * 512:(n2 + 1) * 512],
                                    op0=AL.mult, op1=AL.mult)
                                fin = p_fin.tile([128, 512], BF16, tag="fin",
                                                 name=f"fin{m2}_{h}{n2}")
                                e2.tensor_add(
                                    fin, rkv,
                                    x2[m2][:, h * TH + n2 * 512:
                                           h * TH + (n2 + 1) * 512])
                                fins.append(fin)
                            for j in range(4):
                                tb = h * 8 + n2 * 4 + j
                                pt = ps_t.tile([128, C], BF16, tag="ptr")
                                for m2 in range(NCB):
                                    nc.tensor.transpose(
                                        pt[:, m2 * 128:(m2 + 1) * 128],
                                        fins[m2][:, j * 128:(j + 1) * 128],
                                        identb)
                                st = p_stage.tile([128, C], F32, tag="st")
                                if j % 2 == 0:
                                    nc.scalar.copy(st, pt)
                                else:
                                    nc.vector.tensor_copy(out=st, in_=pt)
                                nc.sync.dma_start(
                                    out=out_d[tb * 128:(tb + 1) * 128, :],
                                    in_=st)
        es_z2.close()
        es_x2.close()
        es_wff.close()
    nc.finalize()
    return nc


_PROG = None


def _get_prog():
    global _PROG
    if _PROG is None:
        nc = bacc.Bacc()
        _PROG = _emit(nc)
    return _PROG


def _tiles_T(w):
    """W [Co, Ci] -> blocks[kb, mb] = W.T tile [128, 128]."""
    co, ci = w.shape
    wt = np.ascontiguousarray(w.T)
    return wt.reshape(ci // 128, 128, co // 128, 128).transpose(0, 2, 1, 3)


def _pack_dr_ab(wa, wb, scale):
    """Folded-lerp DR pack: cols (j, m, s, i) of [128,128] blocks, fp8."""
    f8 = ml_dtypes.float8_e4m3fn
    ta = _tiles_T(wa * scale)
    tb = _tiles_T(wb * scale)
    cols = []
    for j in range(4):
        for m in range(8):
            for s, t_ in ((0, ta), (1, tb)):
                for i in range(2):
                    cols.append(t_[2 * j + i, m])
    return np.ascontiguousarray(np.concatenate(cols, axis=1)).astype(f8)


def _pack_dr(w, scale, kb_pairs, mb_n):
    """Plain DR pack: cols (jp, m, i), fp8. w [Co, Ci]."""
    f8 = ml_dtypes.float8_e4m3fn
    t_ = _tiles_T(w * scale)
    cols = []
    for jp in range(kb_pairs):
        for m in range(mb_n):
            for i in range(2):
                cols.append(t_[2 * jp + i, m])
    return np.ascontiguousarray(np.concatenate(cols, axis=1)).astype(f8)


def _pack_bf(w, outer_n, inner_n):
    """bf16 pack: cols (outer, inner): outer = out-block, inner = k-block."""
    bf = ml_dtypes.bfloat16
    t_ = _tiles_T(w)
    cols = []
    for o in range(outer_n):
        for i in range(inner_n):
            cols.append(t_[i, o])
    return np.ascontiguousarray(np.concatenate(cols, axis=1)).astype(bf)


def _prep_inputs(x, ln1_g, ln1_b, ln2_g, ln2_b, time_decay, time_first,
                 tmk, tmv, tmr, Wk, Wv, Wr, Wo, f_tmk, f_tmr, fWk, fWr, fWv):
    f32 = np.float32
    x = np.asarray(x, f32)
    g1 = np.asarray(ln1_g, f32)
    b1 = np.asarray(ln1_b, f32)
    g2 = np.asarray(ln2_g, f32)
    b2 = np.asarray(ln2_b, f32)
    td = np.asarray(time_decay, np.float64)
    tf = np.asarray(time_first, np.float64)
    tmk = np.asarray(tmk, f32).reshape(C)
    tmv = np.asarray(tmv, f32).reshape(C)
    tmr = np.asarray(tmr, f32).reshape(C)
    ftmk = np.asarray(f_tmk, f32).reshape(C)
    ftmr = np.asarray(f_tmr, f32).reshape(C)
    assert np.array_equal(ftmk, ftmr), "kernel assumes f_tmk == f_tmr"
    assert not b1.any() and not b2.any(), "kernel assumes zero LN biases"
    Wk = np.asarray(Wk, f32) * g1[None, :]
    Wv = np.asarray(Wv, f32) * g1[None, :]
    Wr = np.asarray(Wr, f32) * g1[None, :]
    Wo = np.asarray(Wo, f32)
    fWk1 = np.asarray(fWk, f32) * g2[None, :]
    fWr1 = np.asarray(fWr, f32) * g2[None, :]
    fWv = np.asarray(fWv, f32)

    wbar = np.exp(-np.exp(td)).astype(f32)
    eu = np.exp(tf).astype(f32)

    def packc(v):
        return np.asarray(v, f32).reshape(NCB, 128).T

    def wsc(*ws):
        return float(WMAX / max(np.abs(w).max() for w in ws))

    WkA, WkB = Wk * tmk[None, :], Wk * (1 - tmk)[None, :]
    WvA, WvB = Wv * tmv[None, :], Wv * (1 - tmv)[None, :]
    WrA, WrB = Wr * tmr[None, :], Wr * (1 - tmr)[None, :]
    sWk = wsc(WkA, WkB)
    sWv = wsc(WvA, WvB)
    sWr = wsc(WrA, WrB)
    sWo = wsc(Wo)
    sFV = wsc(fWv)

    cst = np.zeros((128, NCOLS), f32)
    cst[:, C_WBAR:C_WBAR + 8] = packc(wbar)
    cst[:, C_EU:C_EU + 8] = packc(eu)
    cst[:, C_FTM:C_FTM + 8] = packc(ftmk)
    cst[:, C_FTM1:C_FTM1 + 8] = packc(1 - ftmk)
    cst[:, C_EPS] = EPS
    cst[:, C_ZERO] = 0.0
    cst[:, C_CV] = SS / (sWv * SZ)
    cst[:, C_CO] = 1.0 / (sWo * SS)
    cst[:, C_CFV] = 1.0 / (sFV * SKK)
    cst[:, C_CK] = 1.0 / (sWk * SZ)
    cst[:, C_CR] = 1.0 / (sWr * SZ)

    bf = ml_dtypes.bfloat16
    shared = {
        "wk8": _pack_dr_ab(WkA, WkB, sWk),
        "wv8": _pack_dr_ab(WvA, WvB, sWv),
        "wr8": _pack_dr_ab(WrA, WrB, sWr),
        "wo8": _pack_dr(Wo, sWo, 4, 8),
        "fwk16": _pack_bf(fWk1 * np.sqrt(SKK), NHB, NCB),
        "fwr16": _pack_bf(fWr1, NCB, NCB),
        "fwv8": _pack_dr(fWv, sFV, 16, 8),
        "cst": cst,
        "identb": np.eye(128, dtype=bf),
        "ones1b": np.ones((128, 1), bf),
        "onesbr": np.ones((1, 128), f32),
    }
    in_maps = [dict(shared, x=np.ascontiguousarray(x[b])) for b in range(B)]
    return in_maps


def _run(in_maps, trace=False, **kw):
    nc = _get_prog()
    res = run_bass_kernel_spmd(nc, in_maps, core_ids=list(range(B)),
                               trace=trace, **kw)
    out = np.stack([np.asarray(res.results[b]["out"]) for b in range(B)],
                   axis=0)
    return out.astype(np.float32), res


def kernel(*a, **kw):
    out, _ = _run(_prep_inputs(*a, **kw))
    return out


def kernel_traced(*a, **kw):
    return _run(_prep_inputs(*a, **kw), trace=True)


if __name__ == "__main__":
    _get_prog()
    print("program built ok")
    from concourse.timeline_sim import TimelineSim
    print(f"HW exec time: {TimelineSim(_get_prog()).simulate():.0f} ns")
